# revision 31
# baseline (speedup 1.0000x reference)
"""Trainium2 Bass kernel for nn_Encoder_79585743995180 (sparse_attention).

Self-contained: hardcodes shapes/sharding.

Sharding (8 cores): core c = (batch n = c//2, head-group hg = c%2 of 8 heads).
Each core reads x[:, n-rows] only (6MB vs 24MB for head-only sharding),
computes q/k/v projections for its 512 dims, rope, main attention with
column-softmax folded into a 1/colsum prescale of the AV moving operand,
memory attention with the mem_mask compacted away on the host (masked slots
gathered; M=512 -> Mp=ceil(count/128)*128), gate folded into wv / vmaug,
and an out_proj partial (contraction over its 512 dims). Host sums 2
partials per batch + bias.

Key device-side structure:
  - AV matmuls are TRANSPOSED (stationary = exp-weights chunk, moving = v),
    so the renormalization denominators land as per-PARTITION scalars and
    the epilogue is cheap tensor_scalar ops (no cross-partition broadcast).
  - attn^T tiles are PE-transposed back to [d, l] for the out_proj.
  - exp on ACT only; rope muls + epilogue on DVE; rope adds, v-prescale on
    Pool (gpsimd); psum->sbuf copies split DVE/ACT.
  - rope partner-swap via 4 SBUF->SBUF partition-block DMAs per (tensor,
    l-half) on the gpsimd queue.
  - matmul operands fp16 (4x mantissa of bf16 at the same PE rate);
    PSUM accum + softmax denominators fp32; 1/colsum scaled by 64 to keep
    the AV moving operand away from fp16 subnormals.
"""

import numpy as np

import concourse.bacc as bacc
import concourse.mybir as mybir
import concourse.tile as tile
from concourse import bass_utils
from concourse.masks import make_identity

F32 = mybir.dt.float32
F16 = mybir.dt.float16
NPF16 = np.float16
AF = mybir.ActivationFunctionType
MUL = mybir.AluOpType.mult
ADD = mybir.AluOpType.add

L = 1024
S = 1024
N = 4
E = 1024
H = 16
D = 64
M = 512
NC = 8
HPC = 8            # heads per core
DC = HPC * D       # 512 dims per core

_COMPILED = {}


def _build(MC, dbg=False):
    """MC = number of 128-slot memory chunks after mask compaction."""
    Mp = MC * 128
    nc = bacc.Bacc("TRN2", target_bir_lowering=False, debug=False)

    # ---- DRAM I/O (x and cos/sin pre-rearranged on host so every DMA is
    # contiguous per partition) ----
    xqT = nc.dram_tensor("xqT", [2, 128, 8, 512], F16, kind="ExternalInput").ap()
    xkT = nc.dram_tensor("xkT", [2, 128, 8, 512], F16, kind="ExternalInput").ap()
    xvT = nc.dram_tensor("xvT", [2, 128, 8, 512], F16, kind="ExternalInput").ap()
    wqd = nc.dram_tensor("wqd", [128, 8, DC], F16, kind="ExternalInput").ap()
    wkd = nc.dram_tensor("wkd", [128, 8, DC], F16, kind="ExternalInput").ap()
    wvd = nc.dram_tensor("wvd", [128, 8, DC], F16, kind="ExternalInput").ap()
    wod = nc.dram_tensor("wod", [128, 4, E], F16, kind="ExternalInput").ap()
    cosq = nc.dram_tensor("cosq", [2, 128, 4, 512], F16,
                          kind="ExternalInput").ap()
    sinq = nc.dram_tensor("sinq", [2, 128, 4, 512], F16,
                          kind="ExternalInput").ap()
    cosk = nc.dram_tensor("cosk", [2, 128, 4, 512], F16,
                          kind="ExternalInput").ap()
    sink = nc.dram_tensor("sink", [2, 128, 4, 512], F16,
                          kind="ExternalInput").ap()
    kmemd = nc.dram_tensor("kmemd", [128, 4, Mp], F16, kind="ExternalInput").ap()
    vmagd = nc.dram_tensor("vmagd", [128, MC, HPC, 65], F16,
                           kind="ExternalInput").ap()
    outd = nc.dram_tensor("outd", [L, E], F16, kind="ExternalOutput").ap()
    dbg_t = {}
    if dbg:
        for nm, shp in (("dbg_qT", [128, 4, L]), ("dbg_kT", [128, 4, L]),
                        ("dbg_v", [128, 8, HPC, 65]),
                        ("dbg_attnT0", [128, HPC * D]),
                        ("dbg_attn", [128, 4, L])):
            dbg_t[nm] = nc.dram_tensor(nm, shp, F16, kind="ExternalOutput").ap()

    with tile.TileContext(nc) as tc:
        with (
            tc.tile_pool(name="const", bufs=1) as const,
            tc.tile_pool(name="big16", bufs=7) as big16,    # 8KB tiles
            tc.tile_pool(name="csp", bufs=8) as csp,        # cos/sin 4KB
            tc.tile_pool(name="rawp", bufs=5) as rawp,      # 4KB tiles
            tc.tile_pool(name="scr", bufs=2) as scrp,       # colsum scratch
            tc.tile_pool(name="qkrot", bufs=1) as qkrot,
            tc.tile_pool(name="vsb", bufs=1) as vsb,
            tc.tile_pool(name="wxm", bufs=2) as wxmp,
            tc.tile_pool(name="vs", bufs=2) as vsp,
            tc.tile_pool(name="attnT", bufs=1) as attnTp,
            tc.tile_pool(name="attns", bufs=1) as attnsp,
            tc.tile_pool(name="small", bufs=8) as small,
            tc.tile_pool(name="tmp64", bufs=2) as tmp64,
            tc.tile_pool(name="ostage", bufs=2) as ostage,
            tc.tile_pool(name="pbig", bufs=2, space="PSUM") as pbig,
            tc.tile_pool(name="psmall", bufs=4, space="PSUM") as psmall,
        ):
            # ---- constants; two hw queues, critical-path-first order ----
            w_sb = {}
            xs = {}
            cs = {}
            for name, wsrc in (("q", wqd), ("k", wkd), ("v", wvd)):
                w_sb[name] = const.tile([128, 8, DC], F16, tag=f"w_{name}",
                                        name=f"w{name}")
            kmem_sb = const.tile([128, 4, Mp], F16)
            vmaug_sb = const.tile([128, MC, HPC, 65], F16)
            wo_sb = const.tile([128, 4, E], F16)
            for name, src in (("q", xqT), ("k", xkT), ("v", xvT)):
                for lc in range(2):
                    xs[name, lc] = big16.tile([128, 8, 512], F16, tag="xs",
                                              name=f"x{name}{lc}")
            for nm in ("cq", "sq", "ck", "sk"):
                for lc in range(2):
                    cs[nm, lc] = csp.tile([128, 4, 512], F16, tag="cs",
                                          name=f"{nm}{lc}")
            # sync queue: q-path then v-path
            nc.sync.dma_start(out=w_sb["q"], in_=wqd)
            nc.sync.dma_start(out=xs["q", 0], in_=xqT[0])
            nc.sync.dma_start(out=xs["q", 1], in_=xqT[1])
            for lc in range(2):
                nc.sync.dma_start(out=cs["cq", lc], in_=cosq[lc])
                nc.sync.dma_start(out=cs["sq", lc], in_=sinq[lc])
            nc.sync.dma_start(out=w_sb["v"], in_=wvd)
            nc.sync.dma_start(out=xs["v", 0], in_=xvT[0])
            nc.sync.dma_start(out=xs["v", 1], in_=xvT[1])
            # scalar queue: k-path, mem + out consts
            nc.scalar.dma_start(out=w_sb["k"], in_=wkd)
            nc.scalar.dma_start(out=xs["k", 0], in_=xkT[0])
            nc.scalar.dma_start(out=xs["k", 1], in_=xkT[1])
            for lc in range(2):
                nc.scalar.dma_start(out=cs["ck", lc], in_=cosk[lc])
                nc.scalar.dma_start(out=cs["sk", lc], in_=sink[lc])
            nc.scalar.dma_start(out=kmem_sb, in_=kmemd)
            nc.scalar.dma_start(out=vmaug_sb, in_=vmagd)
            nc.scalar.dma_start(out=wo_sb, in_=wod)
            ident = const.tile([128, 128], F16)
            make_identity(nc, ident)

            # PE warmup: junk matmuls keep the clock ramping while the
            # first input DMAs land (p-state reaches max after ~3us busy)
            pwarm = pbig.tile([128, 128], F32, tag="pb", name="pwarm")
            for i in range(24):
                nc.tensor.matmul(pwarm, ident, ident, start=True, stop=True)

            # persistent activation tiles
            qT = qkrot.tile([128, 4, L], F16, name="qT")
            kT = qkrot.tile([128, 4, L], F16, name="kT")
            v_sb = vsb.tile([128, 8, HPC, 65], F16, name="v_sb")
            nc.gpsimd.memset(v_sb[:, :, :, 64:65], 1.0)
            attnT = [attnTp.tile([128, HPC * D], F16, name=f"aT{lc}")
                     for lc in range(8)]
            attn_sb = attnsp.tile([128, 4, L], F16, name="attn_sb")

            # ---- projections + rope (q, k) ----
            # psum->f16 copies on ACT (idle early), rope mul/add on DVE
            for name, dest in (("q", qT), ("k", kT)):
                cn = "cq" if name == "q" else "ck"
                sn = "sq" if name == "q" else "sk"
                for lc in range(2):
                    ls = slice(lc * 512, (lc + 1) * 512)
                    raw = rawp.tile([128, 4, 512], F16, tag="raw")
                    for hc in range(4):
                        ps = pbig.tile([128, 512], F32, tag="pb")
                        for kc in range(8):
                            nc.tensor.matmul(
                                ps, w_sb[name][:, kc,
                                               hc * 128:(hc + 1) * 128],
                                xs[name, lc][:, kc, :],
                                start=(kc == 0), stop=(kc == 7))
                        nc.scalar.activation(raw[:, hc, :], ps, AF.Copy)
                    # partner swap (+-32 within each 64 block) via gpsimd DMA
                    sw = rawp.tile([128, 4, 512], F16, tag="raw")
                    for b in (0, 64):
                        nc.gpsimd.dma_start(
                            out=sw[b:b + 32], in_=raw[b + 32:b + 64])
                        nc.gpsimd.dma_start(
                            out=sw[b + 32:b + 64], in_=raw[b:b + 32])
                    # rope: dest = raw*cos + sw*sin_signed
                    t1 = rawp.tile([128, 4, 512], F16, tag="raw")
                    nc.vector.tensor_mul(t1, raw, cs[cn, lc])
                    t2 = rawp.tile([128, 4, 512], F16, tag="raw")
                    nc.vector.tensor_mul(t2, sw, cs[sn, lc])
                    nc.vector.tensor_add(dest[:, :, ls], t1, t2)

            def emit_vproj():
                # v projection (s on partitions); copies on DVE
                for sc in range(8):
                    lc, slo = sc // 4, (sc % 4) * 128
                    ps = pbig.tile([128, 512], F32, tag="pb")
                    for kc in range(8):
                        nc.tensor.matmul(
                            ps, xs["v", lc][:, kc, slo:slo + 128],
                            w_sb["v"][:, kc, :],
                            start=(kc == 0), stop=(kc == 7))
                    nc.vector.tensor_copy(v_sb[:, sc, :, 0:64], ps)

            # ---- attention heads ----
            def emit_qk_exp(h):
                """QK + exp + 1/colsum-prescaled v for head h."""
                hp, base = h // 2, 64 * (h % 2)
                colsum = small.tile([128, 8], F32, tag="cs")
                wxA = big16.tile([128, 4, L], F16, tag="xs", name=f"wxA{h}")
                wxB = big16.tile([128, 4, L], F16, tag="xs", name=f"wxB{h}")
                for sc in range(8):
                    pw = pbig.tile([128, 1024], F32, tag="pb")
                    for lc in range(2):
                        nc.tensor.matmul(
                            pw[:, lc * 512:(lc + 1) * 512],
                            kT[base:base + 64, hp, sc * 128:(sc + 1) * 128],
                            qT[base:base + 64, hp, lc * 512:(lc + 1) * 512],
                            start=True, stop=True)
                    wx = (wxA if sc < 4 else wxB)
                    nc.scalar.activation(wx[:, sc % 4, :], pw, AF.Exp)
                    # colsum on Pool (keeps the ACT stream pure exp)
                    scr = scrp.tile([128, 1024], F16, tag="scr")
                    nc.vector.tensor_scalar(
                        scr, wx[:, sc % 4, :], 1.0, None, op0=MUL, op1=ADD,
                        accum_out=colsum[:, sc:sc + 1])
                rcall = small.tile([128, 8], F32, tag="cs")
                nc.vector.reciprocal_approx_fast(rcall, colsum)
                return wxA, wxB, rcall

            def emit_vs(h, rcall):
                vs = vsp.tile([128, 8, 65], F16, tag="vs")
                for sc in range(8):
                    nc.gpsimd.tensor_scalar(
                        vs[:, sc, :], v_sb[:, sc, h, :],
                        rcall[:, sc:sc + 1], 64.0, op0=MUL, op1=MUL)
                return vs

            def emit_mem_qk_exp(h):
                hp, base = h // 2, 64 * (h % 2)
                wxm = wxmp.tile([128, MC, L], F16, tag="wxm")
                for mc in range(MC):
                    pw = pbig.tile([128, 1024], F32, tag="pb")
                    for lc in range(2):
                        nc.tensor.matmul(
                            pw[:, lc * 512:(lc + 1) * 512],
                            kmem_sb[base:base + 64, hp,
                                    mc * 128:(mc + 1) * 128],
                            qT[base:base + 64, hp, lc * 512:(lc + 1) * 512],
                            start=True, stop=True)
                    nc.scalar.activation(wxm[:, mc, :], pw, AF.Exp)
                return wxm

            def emit_avt_epilogue(h, wxA, wxB, vs, wxm):
                pms = [psmall.tile([128, 4, 128], F32, tag="pm",
                                   name=f"pm{g}") for g in range(2)]
                pmems = [psmall.tile([128, 4, 128], F32, tag="pm",
                                     name=f"pmem{g}") for g in range(2)]
                for lc in range(8):
                    pt = pms[lc // 4][:, lc % 4, 0:65]
                    for sc in range(8):
                        wx = (wxA if sc < 4 else wxB)
                        nc.tensor.matmul(
                            pt, wx[:, sc % 4, lc * 128:(lc + 1) * 128],
                            vs[:, sc, :], start=(sc == 0), stop=(sc == 7))
                for lc in range(8):
                    pt = pmems[lc // 4][:, lc % 4, 0:65]
                    for mc in range(MC):
                        nc.tensor.matmul(
                            pt, wxm[:, mc, lc * 128:(lc + 1) * 128],
                            vmaug_sb[:, mc, h, :],
                            start=(mc == 0), stop=(mc == MC - 1))
                # epilogue: attnT[lc][:, h*64:+64] =
                #   pmain[:, :64]/D1 + pmem[:, :64]/D2   (per-partition)
                for g in range(2):
                    rc1 = small.tile([128, 4, 1], F32, tag="rc")
                    rc2 = small.tile([128, 4, 1], F32, tag="rc")
                    nc.vector.reciprocal_approx_fast(
                        rc1, pms[g][:, :, 64:65])
                    nc.vector.reciprocal_approx_fast(
                        rc2, pmems[g][:, :, 64:65])
                    for j in range(4):
                        lc = g * 4 + j
                        tmp = tmp64.tile([128, 64], F16, tag="t64")
                        nc.vector.tensor_scalar_mul(
                            tmp, pmems[g][:, j, 0:64], rc2[:, j, 0:1])
                        nc.vector.scalar_tensor_tensor(
                            out=attnT[lc][:, h * 64:(h + 1) * 64],
                            in0=pms[g][:, j, 0:64],
                            scalar=rc1[:, j, 0:1],
                            in1=tmp, op0=MUL, op1=ADD)

            def emit_transpose(hpair, lcs=range(8)):
                d0 = hpair * 128
                for lc in lcs:
                    ptr = psmall.tile([128, 128], F16, tag="pm", name="ptr")
                    nc.tensor.transpose(
                        ptr, attnT[lc][:, d0:d0 + 128], ident)
                    nc.vector.tensor_copy(
                        attn_sb[:, hpair, lc * 128:(lc + 1) * 128], ptr)

            # software pipeline over heads: PE order within an iteration is
            # memQK(h), AVT(h), AVTm(h), QK(h+1) so AVT never sits behind
            # the exp-paced QK of the next head. v-proj runs under exp(0).
            wxA, wxB, rcall = emit_qk_exp(0)
            emit_vproj()
            wx_cur = (wxA, wxB, emit_vs(0, rcall))
            for h in range(HPC):
                wxm = emit_mem_qk_exp(h)
                emit_avt_epilogue(h, *wx_cur, wxm)
                if h + 1 < HPC:
                    wxA, wxB, rcall = emit_qk_exp(h + 1)
                    wx_cur = (wxA, wxB, emit_vs(h + 1, rcall))
                if h % 2 == 1 and h < HPC - 1:
                    emit_transpose(h // 2)

            if dbg:
                nc.sync.dma_start(out=dbg_t["dbg_qT"], in_=qT)
                nc.sync.dma_start(out=dbg_t["dbg_kT"], in_=kT)
                nc.sync.dma_start(out=dbg_t["dbg_v"], in_=v_sb)
                nc.sync.dma_start(out=dbg_t["dbg_attnT0"], in_=attnT[0])
                nc.sync.dma_start(out=dbg_t["dbg_attn"], in_=attn_sb)

            # ---- out_proj: out[l, e] = sum_d attn[d, l] * wo[d, e] ----
            # last head pair's transpose interleaved per l-chunk
            dmaq = [nc.sync, nc.gpsimd, nc.sync, nc.gpsimd]
            for lc in range(8):
                emit_transpose(3, [lc])
                for ec in range(2):
                    po = pbig.tile([128, 512], F32, tag="pb")
                    for dc in range(4):
                        nc.tensor.matmul(
                            po, attn_sb[:, dc, lc * 128:(lc + 1) * 128],
                            wo_sb[:, dc, ec * 512:(ec + 1) * 512],
                            start=(dc == 0), stop=(dc == 3))
                    so = ostage.tile([128, 512], F16, tag="so")
                    if (lc * 2 + ec) % 2 == 0:
                        nc.scalar.activation(so, po, AF.Copy)
                    else:
                        nc.vector.tensor_copy(so, po)
                    dmaq[(lc * 2 + ec) % 4].dma_start(
                        out=outd[lc * 128:(lc + 1) * 128,
                                 ec * 512:(ec + 1) * 512], in_=so)
    nc.compile()
    return nc


def _perm64():
    p = np.empty(64, np.int64)
    p[:32] = np.arange(0, 64, 2)
    p[32:] = np.arange(1, 64, 2)
    return p


def _prep_inputs(inputs):
    """Host-side shard prep. Returns (MC, list of per-core input dicts)."""
    f = np.float32
    query = np.asarray(inputs["query"], f)
    key = np.asarray(inputs["key"], f)
    value = np.asarray(inputs["value"], f)
    W = np.asarray(inputs["in_proj_weight"], f)
    wo = np.asarray(inputs["out_proj_weight"], f)
    qp = np.asarray(inputs["qp"], f)
    kvp = np.asarray(inputs["kvp"], f)
    k_mem = np.asarray(inputs["k_mem"], f)
    v_mem = np.asarray(inputs["v_mem"], f)
    gate = np.asarray(inputs["gate_attn"], f)
    mask = np.asarray(inputs["mem_mask"])

    g = 1.0 / (1.0 + np.exp(-gate))
    p64 = _perm64()
    sgn = np.tile(np.concatenate([np.full(32, -1.0, f), np.full(32, 1.0, f)]),
                  HPC)[:, None]

    midx = [np.nonzero(mask[n])[0] for n in range(N)]
    MC = max(1, (max(len(m) for m in midx) + 127) // 128)
    Mp = MC * 128

    def xdev(a, n):
        # (E, L) -> [2(lc), 128(p), 8(kc), 512(r)] contiguous for DMA
        t = a[:, n, :].T.reshape(8, 128, 2, 512)
        return np.ascontiguousarray(t.transpose(2, 1, 0, 3)).astype(NPF16)

    # per-batch x slices (shared by the two cores of a batch)
    xq = [xdev(query, n) for n in range(N)]
    xk = [xdev(key, n) for n in range(N)]
    xv = [xdev(value, n) for n in range(N)]

    def dev3(a, npart=128):
        """(Ptot, F) -> (128, Ptot//128, F) partition-chunked layout."""
        ptot = a.shape[0]
        return np.ascontiguousarray(
            a.reshape(ptot // npart, npart, -1).transpose(1, 0, 2))

    def csdev(a):
        """(512, 1024) -> [2(lc), 128, 4(hc), 512] contiguous."""
        return np.ascontiguousarray(
            a.reshape(4, 128, 2, 512).transpose(2, 1, 0, 3)).astype(NPF16)

    in_maps = []
    for c in range(NC):
        n, hg = c // 2, c % 2
        heads = np.arange(hg * 8, hg * 8 + 8)
        dims_plain = np.concatenate([h * 64 + np.arange(64) for h in heads])
        dims_perm = np.concatenate([h * 64 + p64 for h in heads])

        wq = (W[:E][dims_perm] * np.float32(D ** -0.5))
        wk = W[E:2 * E][dims_perm]
        gv = np.repeat(1.0 - g[heads], 64).astype(f)
        wv = W[2 * E:][dims_plain] * gv[:, None]

        cq = qp[n][:, dims_perm, 0].T
        sq = qp[n][:, dims_perm, 1].T * sgn
        ck = kvp[n][:, dims_perm, 0].T
        sk = kvp[n][:, dims_perm, 1].T * sgn

        mi = midx[n]
        kmem = np.zeros((DC, Mp), f)
        kmem[:, :len(mi)] = k_mem[n][dims_perm][:, mi]
        vma = np.zeros((Mp, HPC, 65), f)
        for hl, h in enumerate(heads):
            vma[:len(mi), hl, :64] = (
                v_mem[n, h * 64:h * 64 + 64][:, mi].T * g[h])
            vma[:len(mi), hl, 64] = 1.0

        in_maps.append({
            "xqT": xq[n], "xkT": xk[n], "xvT": xv[n],
            "wqd": dev3(wq.T).astype(NPF16),
            "wkd": dev3(wk.T).astype(NPF16),
            "wvd": dev3(wv.T).astype(NPF16),
            "wod": dev3(wo[:, dims_plain].T).astype(NPF16),
            "cosq": csdev(cq), "sinq": csdev(sq),
            "cosk": csdev(ck), "sink": csdev(sk),
            "kmemd": dev3(kmem).astype(NPF16),
            "vmagd": np.ascontiguousarray(
                vma.reshape(MC, 128, HPC, 65).transpose(1, 0, 2, 3)
            ).astype(NPF16),
        })
    return MC, in_maps


def kernel(**inputs):
    MC, in_maps = _prep_inputs(inputs)
    if MC not in _COMPILED:
        _COMPILED[MC] = _build(MC)
    nc = _COMPILED[MC]
    _COMPILED["last"] = nc
    res = bass_utils.run_bass_kernel_spmd(nc, in_maps, core_ids=list(range(NC)))
    out = np.zeros((L, N, E), f := np.float32)
    for n in range(N):
        out[:, n, :] = (res.results[2 * n]["outd"].astype(f)
                        + res.results[2 * n + 1]["outd"].astype(f))
    out += np.asarray(inputs["out_proj_bias"], f)
    return out


# revision 32
# speedup vs baseline: 1.1144x; 1.1144x over previous
"""Trainium2 Bass kernel for nn_Encoder_79585743995180 (sparse_attention).

Self-contained: hardcodes shapes/sharding.

Sharding (8 cores): core c = (batch n = c//2, head-group hg = c%2 of 8 heads).
Each core reads x[:, n-rows] only (6MB vs 24MB for head-only sharding),
computes q/k/v projections for its 512 dims, rope, main attention with
column-softmax folded into a 1/colsum prescale of the AV moving operand,
memory attention with the mem_mask compacted away on the host (masked slots
gathered; M=512 -> Mp=ceil(count/128)*128), gate folded into wv / vmaug,
and an out_proj partial (contraction over its 512 dims). Host sums 2
partials per batch + bias.

Key device-side structure:
  - AV matmuls are TRANSPOSED (stationary = exp-weights chunk, moving = v),
    so the renormalization denominators land as per-PARTITION scalars and
    the epilogue is cheap tensor_scalar ops (no cross-partition broadcast).
  - attn^T tiles are PE-transposed back to [d, l] for the out_proj.
  - exp on ACT only; rope muls + epilogue on DVE; rope adds, v-prescale on
    Pool (gpsimd); psum->sbuf copies split DVE/ACT.
  - rope partner-swap via 4 SBUF->SBUF partition-block DMAs per (tensor,
    l-half) on the gpsimd queue.
  - matmul operands fp16 (4x mantissa of bf16 at the same PE rate);
    PSUM accum + softmax denominators fp32; 1/colsum scaled by 64 to keep
    the AV moving operand away from fp16 subnormals.
"""

import numpy as np

import concourse.bacc as bacc
import concourse.mybir as mybir
import concourse.tile as tile
from concourse import bass_utils
from concourse.masks import make_identity

F32 = mybir.dt.float32
F16 = mybir.dt.float16
NPF16 = np.float16
AF = mybir.ActivationFunctionType
MUL = mybir.AluOpType.mult
ADD = mybir.AluOpType.add

L = 1024
S = 1024
N = 4
E = 1024
H = 16
D = 64
M = 512
NC = 8
HPC = 8            # heads per core
DC = HPC * D       # 512 dims per core

_COMPILED = {}


def _build(MC, dbg=False):
    """MC = number of 128-slot memory chunks after mask compaction."""
    Mp = MC * 128
    nc = bacc.Bacc("TRN2", target_bir_lowering=False, debug=False)

    # ---- DRAM I/O (x and cos/sin pre-rearranged on host so every DMA is
    # contiguous per partition) ----
    xqT = nc.dram_tensor("xqT", [2, 128, 8, 512], F16, kind="ExternalInput").ap()
    xkT = nc.dram_tensor("xkT", [2, 128, 8, 512], F16, kind="ExternalInput").ap()
    xvT = nc.dram_tensor("xvT", [2, 128, 8, 512], F16, kind="ExternalInput").ap()
    wqd = nc.dram_tensor("wqd", [128, 8, DC], F16, kind="ExternalInput").ap()
    wkd = nc.dram_tensor("wkd", [128, 8, DC], F16, kind="ExternalInput").ap()
    wvd = nc.dram_tensor("wvd", [128, 8, DC], F16, kind="ExternalInput").ap()
    wod = nc.dram_tensor("wod", [128, 4, E], F16, kind="ExternalInput").ap()
    cosq = nc.dram_tensor("cosq", [2, 128, 4, 512], F16,
                          kind="ExternalInput").ap()
    sinq = nc.dram_tensor("sinq", [2, 128, 4, 512], F16,
                          kind="ExternalInput").ap()
    cosk = nc.dram_tensor("cosk", [2, 128, 4, 512], F16,
                          kind="ExternalInput").ap()
    sink = nc.dram_tensor("sink", [2, 128, 4, 512], F16,
                          kind="ExternalInput").ap()
    kmemd = nc.dram_tensor("kmemd", [128, 4, Mp], F16, kind="ExternalInput").ap()
    vmagd = nc.dram_tensor("vmagd", [128, MC, HPC, 65], F16,
                           kind="ExternalInput").ap()
    outd = nc.dram_tensor("outd", [L, E], F16, kind="ExternalOutput").ap()
    dbg_t = {}
    if dbg:
        for nm, shp in (("dbg_qT", [128, 4, L]), ("dbg_kT", [128, 4, L]),
                        ("dbg_v", [128, 8, HPC, 65]),
                        ("dbg_attnT0", [128, HPC * D]),
                        ("dbg_attn", [128, 4, L])):
            dbg_t[nm] = nc.dram_tensor(nm, shp, F16, kind="ExternalOutput").ap()

    with tile.TileContext(nc) as tc:
        with (
            tc.tile_pool(name="const", bufs=1) as const,
            tc.tile_pool(name="big16", bufs=7) as big16,    # 8KB tiles
            tc.tile_pool(name="csp", bufs=8) as csp,        # cos/sin 4KB
            tc.tile_pool(name="rawp", bufs=5) as rawp,      # 4KB tiles
            tc.tile_pool(name="scr", bufs=2) as scrp,       # colsum scratch
            tc.tile_pool(name="qkrot", bufs=1) as qkrot,
            tc.tile_pool(name="vsb", bufs=1) as vsb,
            tc.tile_pool(name="wxm", bufs=2) as wxmp,
            tc.tile_pool(name="vs", bufs=2) as vsp,
            tc.tile_pool(name="attnT", bufs=1) as attnTp,
            tc.tile_pool(name="attns", bufs=1) as attnsp,
            tc.tile_pool(name="small", bufs=8) as small,
            tc.tile_pool(name="tmp64", bufs=2) as tmp64,
            tc.tile_pool(name="ostage", bufs=2) as ostage,
            tc.tile_pool(name="pbig", bufs=2, space="PSUM") as pbig,
            tc.tile_pool(name="psmall", bufs=4, space="PSUM") as psmall,
        ):
            # ---- constants; two hw queues, critical-path-first order ----
            w_sb = {}
            xs = {}
            cs = {}
            for name, wsrc in (("q", wqd), ("k", wkd), ("v", wvd)):
                w_sb[name] = const.tile([128, 8, DC], F16, tag=f"w_{name}",
                                        name=f"w{name}")
            kmem_sb = const.tile([128, 4, Mp], F16)
            vmaug_sb = const.tile([128, MC, HPC, 65], F16)
            wo_sb = const.tile([128, 4, E], F16)
            for name, src in (("q", xqT), ("k", xkT), ("v", xvT)):
                for lc in range(2):
                    xs[name, lc] = big16.tile([128, 8, 512], F16, tag="xs",
                                              name=f"x{name}{lc}")
            for nm in ("cq", "sq", "ck", "sk"):
                for lc in range(2):
                    cs[nm, lc] = csp.tile([128, 4, 512], F16, tag="cs",
                                          name=f"{nm}{lc}")
            # sync queue: q-path then v-path
            nc.sync.dma_start(out=w_sb["q"], in_=wqd)
            nc.sync.dma_start(out=xs["q", 0], in_=xqT[0])
            nc.sync.dma_start(out=xs["q", 1], in_=xqT[1])
            for lc in range(2):
                nc.sync.dma_start(out=cs["cq", lc], in_=cosq[lc])
                nc.sync.dma_start(out=cs["sq", lc], in_=sinq[lc])
            nc.sync.dma_start(out=w_sb["v"], in_=wvd)
            nc.sync.dma_start(out=xs["v", 0], in_=xvT[0])
            nc.sync.dma_start(out=xs["v", 1], in_=xvT[1])
            # scalar queue: k-path, mem + out consts
            nc.scalar.dma_start(out=w_sb["k"], in_=wkd)
            nc.scalar.dma_start(out=xs["k", 0], in_=xkT[0])
            nc.scalar.dma_start(out=xs["k", 1], in_=xkT[1])
            for lc in range(2):
                nc.scalar.dma_start(out=cs["ck", lc], in_=cosk[lc])
                nc.scalar.dma_start(out=cs["sk", lc], in_=sink[lc])
            nc.scalar.dma_start(out=kmem_sb, in_=kmemd)
            nc.scalar.dma_start(out=vmaug_sb, in_=vmagd)
            nc.scalar.dma_start(out=wo_sb, in_=wod)
            ident = const.tile([128, 128], F16)
            make_identity(nc, ident)

            # PE warmup: junk matmuls keep the clock ramping while the
            # first input DMAs land (p-state reaches max after ~3us busy)
            pwarm = pbig.tile([128, 128], F32, tag="pb", name="pwarm")
            for i in range(24):
                nc.tensor.matmul(pwarm, ident, ident, start=True, stop=True)

            # persistent activation tiles
            qT = qkrot.tile([128, 4, L], F16, name="qT")
            kT = qkrot.tile([128, 4, L], F16, name="kT")
            v_sb = vsb.tile([128, 8, HPC, 65], F16, name="v_sb")
            nc.gpsimd.memset(v_sb[:, :, :, 64:65], 1.0)
            attnT = [attnTp.tile([128, HPC * D], F16, name=f"aT{lc}")
                     for lc in range(8)]
            attn_sb = attnsp.tile([128, 4, L], F16, name="attn_sb")

            # ---- projections + rope (q, k) ----
            # psum->f16 copies on ACT (idle early), rope mul/add on DVE
            for name, dest in (("q", qT), ("k", kT)):
                cn = "cq" if name == "q" else "ck"
                sn = "sq" if name == "q" else "sk"
                for lc in range(2):
                    ls = slice(lc * 512, (lc + 1) * 512)
                    raw = rawp.tile([128, 4, 512], F16, tag="raw")
                    for hc in range(4):
                        ps = pbig.tile([128, 512], F32, tag="pb")
                        for kc in range(8):
                            nc.tensor.matmul(
                                ps, w_sb[name][:, kc,
                                               hc * 128:(hc + 1) * 128],
                                xs[name, lc][:, kc, :],
                                start=(kc == 0), stop=(kc == 7))
                        nc.scalar.activation(raw[:, hc, :], ps, AF.Copy)
                    # partner swap (+-32 within each 64 block) via gpsimd DMA
                    sw = rawp.tile([128, 4, 512], F16, tag="raw")
                    for b in (0, 64):
                        nc.gpsimd.dma_start(
                            out=sw[b:b + 32], in_=raw[b + 32:b + 64])
                        nc.gpsimd.dma_start(
                            out=sw[b + 32:b + 64], in_=raw[b:b + 32])
                    # rope: dest = raw*cos + sw*sin_signed
                    t1 = rawp.tile([128, 4, 512], F16, tag="raw")
                    nc.vector.tensor_mul(t1, raw, cs[cn, lc])
                    t2 = rawp.tile([128, 4, 512], F16, tag="raw")
                    nc.vector.tensor_mul(t2, sw, cs[sn, lc])
                    nc.vector.tensor_add(dest[:, :, ls], t1, t2)

            def emit_vproj():
                # v projection (s on partitions); copies on DVE
                for sc in range(8):
                    lc, slo = sc // 4, (sc % 4) * 128
                    ps = pbig.tile([128, 512], F32, tag="pb")
                    for kc in range(8):
                        nc.tensor.matmul(
                            ps, xs["v", lc][:, kc, slo:slo + 128],
                            w_sb["v"][:, kc, :],
                            start=(kc == 0), stop=(kc == 7))
                    nc.vector.tensor_copy(v_sb[:, sc, :, 0:64], ps)

            # ---- attention heads ----
            def emit_qk_exp(h):
                """QK + exp + 1/colsum-prescaled v for head h."""
                hp, base = h // 2, 64 * (h % 2)
                colsum = small.tile([128, 8], F32, tag="cs")
                wxA = big16.tile([128, 4, L], F16, tag="xs", name=f"wxA{h}")
                wxB = big16.tile([128, 4, L], F16, tag="xs", name=f"wxB{h}")
                for sc in range(8):
                    pw = pbig.tile([128, 1024], F32, tag="pb")
                    for lc in range(2):
                        nc.tensor.matmul(
                            pw[:, lc * 512:(lc + 1) * 512],
                            kT[base:base + 64, hp, sc * 128:(sc + 1) * 128],
                            qT[base:base + 64, hp, lc * 512:(lc + 1) * 512],
                            start=True, stop=True)
                    wx = (wxA if sc < 4 else wxB)
                    nc.scalar.activation(
                        wx[:, sc % 4, :], pw, AF.Exp,
                        accum_out=colsum[:, sc:sc + 1])
                rcall = small.tile([128, 8], F32, tag="cs")
                nc.vector.reciprocal_approx_fast(rcall, colsum)
                return wxA, wxB, rcall

            def emit_vs(h, rcall):
                vs = vsp.tile([128, 8, 65], F16, tag="vs")
                for sc in range(8):
                    nc.gpsimd.tensor_scalar(
                        vs[:, sc, :], v_sb[:, sc, h, :],
                        rcall[:, sc:sc + 1], 64.0, op0=MUL, op1=MUL)
                return vs

            def emit_mem_qk_exp(h):
                hp, base = h // 2, 64 * (h % 2)
                wxm = wxmp.tile([128, MC, L], F16, tag="wxm")
                for mc in range(MC):
                    pw = pbig.tile([128, 1024], F32, tag="pb")
                    for lc in range(2):
                        nc.tensor.matmul(
                            pw[:, lc * 512:(lc + 1) * 512],
                            kmem_sb[base:base + 64, hp,
                                    mc * 128:(mc + 1) * 128],
                            qT[base:base + 64, hp, lc * 512:(lc + 1) * 512],
                            start=True, stop=True)
                    nc.scalar.activation(wxm[:, mc, :], pw, AF.Exp)
                return wxm

            def emit_avt_epilogue(h, wxA, wxB, vs, wxm):
                pms = [psmall.tile([128, 4, 128], F32, tag="pm",
                                   name=f"pm{g}") for g in range(2)]
                pmems = [psmall.tile([128, 4, 128], F32, tag="pm",
                                     name=f"pmem{g}") for g in range(2)]
                for lc in range(8):
                    pt = pms[lc // 4][:, lc % 4, 0:65]
                    for sc in range(8):
                        wx = (wxA if sc < 4 else wxB)
                        nc.tensor.matmul(
                            pt, wx[:, sc % 4, lc * 128:(lc + 1) * 128],
                            vs[:, sc, :], start=(sc == 0), stop=(sc == 7))
                for lc in range(8):
                    pt = pmems[lc // 4][:, lc % 4, 0:65]
                    for mc in range(MC):
                        nc.tensor.matmul(
                            pt, wxm[:, mc, lc * 128:(lc + 1) * 128],
                            vmaug_sb[:, mc, h, :],
                            start=(mc == 0), stop=(mc == MC - 1))
                # epilogue: attnT[lc][:, h*64:+64] =
                #   pmain[:, :64]/D1 + pmem[:, :64]/D2   (per-partition)
                for g in range(2):
                    rc1 = small.tile([128, 4, 1], F32, tag="rc")
                    rc2 = small.tile([128, 4, 1], F32, tag="rc")
                    nc.vector.reciprocal_approx_fast(
                        rc1, pms[g][:, :, 64:65])
                    nc.vector.reciprocal_approx_fast(
                        rc2, pmems[g][:, :, 64:65])
                    for j in range(4):
                        lc = g * 4 + j
                        tmp = tmp64.tile([128, 64], F16, tag="t64")
                        nc.vector.tensor_scalar_mul(
                            tmp, pmems[g][:, j, 0:64], rc2[:, j, 0:1])
                        nc.vector.scalar_tensor_tensor(
                            out=attnT[lc][:, h * 64:(h + 1) * 64],
                            in0=pms[g][:, j, 0:64],
                            scalar=rc1[:, j, 0:1],
                            in1=tmp, op0=MUL, op1=ADD)

            def emit_transpose(hpair, lcs=range(8)):
                d0 = hpair * 128
                for lc in lcs:
                    ptr = psmall.tile([128, 128], F16, tag="pm", name="ptr")
                    nc.tensor.transpose(
                        ptr, attnT[lc][:, d0:d0 + 128], ident)
                    nc.vector.tensor_copy(
                        attn_sb[:, hpair, lc * 128:(lc + 1) * 128], ptr)

            # software pipeline over heads: PE order within an iteration is
            # memQK(h), AVT(h), AVTm(h), QK(h+1) so AVT never sits behind
            # the exp-paced QK of the next head. v-proj runs under exp(0).
            wxA, wxB, rcall = emit_qk_exp(0)
            emit_vproj()
            wx_cur = (wxA, wxB, emit_vs(0, rcall))
            for h in range(HPC):
                wxm = emit_mem_qk_exp(h)
                emit_avt_epilogue(h, *wx_cur, wxm)
                if h + 1 < HPC:
                    wxA, wxB, rcall = emit_qk_exp(h + 1)
                    wx_cur = (wxA, wxB, emit_vs(h + 1, rcall))
                if h % 2 == 1 and h < HPC - 1:
                    emit_transpose(h // 2)

            if dbg:
                nc.sync.dma_start(out=dbg_t["dbg_qT"], in_=qT)
                nc.sync.dma_start(out=dbg_t["dbg_kT"], in_=kT)
                nc.sync.dma_start(out=dbg_t["dbg_v"], in_=v_sb)
                nc.sync.dma_start(out=dbg_t["dbg_attnT0"], in_=attnT[0])
                nc.sync.dma_start(out=dbg_t["dbg_attn"], in_=attn_sb)

            # ---- out_proj: out[l, e] = sum_d attn[d, l] * wo[d, e] ----
            # last head pair's transpose interleaved per l-chunk
            dmaq = [nc.sync, nc.gpsimd, nc.sync, nc.gpsimd]
            for lc in range(8):
                emit_transpose(3, [lc])
                for ec in range(2):
                    po = pbig.tile([128, 512], F32, tag="pb")
                    for dc in range(4):
                        nc.tensor.matmul(
                            po, attn_sb[:, dc, lc * 128:(lc + 1) * 128],
                            wo_sb[:, dc, ec * 512:(ec + 1) * 512],
                            start=(dc == 0), stop=(dc == 3))
                    so = ostage.tile([128, 512], F16, tag="so")
                    if (lc * 2 + ec) % 2 == 0:
                        nc.scalar.activation(so, po, AF.Copy)
                    else:
                        nc.vector.tensor_copy(so, po)
                    dmaq[(lc * 2 + ec) % 4].dma_start(
                        out=outd[lc * 128:(lc + 1) * 128,
                                 ec * 512:(ec + 1) * 512], in_=so)
    nc.compile()
    return nc


def _perm64():
    p = np.empty(64, np.int64)
    p[:32] = np.arange(0, 64, 2)
    p[32:] = np.arange(1, 64, 2)
    return p


def _prep_inputs(inputs):
    """Host-side shard prep. Returns (MC, list of per-core input dicts)."""
    f = np.float32
    query = np.asarray(inputs["query"], f)
    key = np.asarray(inputs["key"], f)
    value = np.asarray(inputs["value"], f)
    W = np.asarray(inputs["in_proj_weight"], f)
    wo = np.asarray(inputs["out_proj_weight"], f)
    qp = np.asarray(inputs["qp"], f)
    kvp = np.asarray(inputs["kvp"], f)
    k_mem = np.asarray(inputs["k_mem"], f)
    v_mem = np.asarray(inputs["v_mem"], f)
    gate = np.asarray(inputs["gate_attn"], f)
    mask = np.asarray(inputs["mem_mask"])

    g = 1.0 / (1.0 + np.exp(-gate))
    p64 = _perm64()
    sgn = np.tile(np.concatenate([np.full(32, -1.0, f), np.full(32, 1.0, f)]),
                  HPC)[:, None]

    midx = [np.nonzero(mask[n])[0] for n in range(N)]
    MC = max(1, (max(len(m) for m in midx) + 127) // 128)
    Mp = MC * 128

    def xdev(a, n):
        # (E, L) -> [2(lc), 128(p), 8(kc), 512(r)] contiguous for DMA
        t = a[:, n, :].T.reshape(8, 128, 2, 512)
        return np.ascontiguousarray(t.transpose(2, 1, 0, 3)).astype(NPF16)

    # per-batch x slices (shared by the two cores of a batch)
    xq = [xdev(query, n) for n in range(N)]
    xk = [xdev(key, n) for n in range(N)]
    xv = [xdev(value, n) for n in range(N)]

    def dev3(a, npart=128):
        """(Ptot, F) -> (128, Ptot//128, F) partition-chunked layout."""
        ptot = a.shape[0]
        return np.ascontiguousarray(
            a.reshape(ptot // npart, npart, -1).transpose(1, 0, 2))

    def csdev(a):
        """(512, 1024) -> [2(lc), 128, 4(hc), 512] contiguous."""
        return np.ascontiguousarray(
            a.reshape(4, 128, 2, 512).transpose(2, 1, 0, 3)).astype(NPF16)

    in_maps = []
    for c in range(NC):
        n, hg = c // 2, c % 2
        heads = np.arange(hg * 8, hg * 8 + 8)
        dims_plain = np.concatenate([h * 64 + np.arange(64) for h in heads])
        dims_perm = np.concatenate([h * 64 + p64 for h in heads])

        wq = (W[:E][dims_perm] * np.float32(D ** -0.5))
        wk = W[E:2 * E][dims_perm]
        gv = np.repeat(1.0 - g[heads], 64).astype(f)
        wv = W[2 * E:][dims_plain] * gv[:, None]

        cq = qp[n][:, dims_perm, 0].T
        sq = qp[n][:, dims_perm, 1].T * sgn
        ck = kvp[n][:, dims_perm, 0].T
        sk = kvp[n][:, dims_perm, 1].T * sgn

        mi = midx[n]
        kmem = np.zeros((DC, Mp), f)
        kmem[:, :len(mi)] = k_mem[n][dims_perm][:, mi]
        vma = np.zeros((Mp, HPC, 65), f)
        for hl, h in enumerate(heads):
            vma[:len(mi), hl, :64] = (
                v_mem[n, h * 64:h * 64 + 64][:, mi].T * g[h])
            vma[:len(mi), hl, 64] = 1.0

        in_maps.append({
            "xqT": xq[n], "xkT": xk[n], "xvT": xv[n],
            "wqd": dev3(wq.T).astype(NPF16),
            "wkd": dev3(wk.T).astype(NPF16),
            "wvd": dev3(wv.T).astype(NPF16),
            "wod": dev3(wo[:, dims_plain].T).astype(NPF16),
            "cosq": csdev(cq), "sinq": csdev(sq),
            "cosk": csdev(ck), "sink": csdev(sk),
            "kmemd": dev3(kmem).astype(NPF16),
            "vmagd": np.ascontiguousarray(
                vma.reshape(MC, 128, HPC, 65).transpose(1, 0, 2, 3)
            ).astype(NPF16),
        })
    return MC, in_maps


def kernel(**inputs):
    MC, in_maps = _prep_inputs(inputs)
    if MC not in _COMPILED:
        _COMPILED[MC] = _build(MC)
    nc = _COMPILED[MC]
    _COMPILED["last"] = nc
    res = bass_utils.run_bass_kernel_spmd(nc, in_maps, core_ids=list(range(NC)))
    out = np.zeros((L, N, E), f := np.float32)
    for n in range(N):
        out[:, n, :] = (res.results[2 * n]["outd"].astype(f)
                        + res.results[2 * n + 1]["outd"].astype(f))
    out += np.asarray(inputs["out_proj_bias"], f)
    return out
